# revision 1
# baseline (speedup 1.0000x reference)
"""Trainium2 Bass kernel for MixformerAttention (sparse attention).

Problem shape (hardcoded from the problem spec):
  x [B=64, N=320, C=768], W_qkv [768, 2304], W_proj [768, 768], b_proj [768]
  H=12 heads, Dh=64, template length L = t_h*t_w = 64, search = 256.

Sharding: data-parallel over batch across 8 NeuronCores (8 batches/core).

Per-core pipeline (batches processed in pairs of 2 -> 640 tokens = 5x128):
  1. DMA x pair -> SBUF, PE-transpose to x^T (C on partitions), cast fp16
  2. QKV^T matmul for q,k  ->  q^T,k^T [Dh, tok] per head (fp16); odd heads
     are DMA-shifted to partition base 0 (matmul operands must not live at
     SBUF partition base 64 - implicit PE row-tiling crashes the exec unit)
     V natural matmul      ->  v [tok, head, 65] with a ones column (fp16)
  3. Per batch: S^T = k q^T per head/key-chunk -> exp (ACT, scale=1/8) -> fp16
     PV: O[q, h-slot] = expS^T.T @ [v|1]  (ones column yields softmax denom);
     query chunks (template 64 | search 128 | search 128), all at psum base 0
     normalize rows by 1/denominator -> attn [tok, C] fp16
  4. attn^T via DMA tile-transposes, proj matmul + bias -> out fp32 -> DMA out

All matmuls use fp16 operands (1 cycle/row on the PE, fp32 PSUM accumulate).
"""

import functools

import numpy as np

import concourse.bacc as bacc
import concourse.mybir as mybir
from concourse.bass_utils import run_bass_kernel_spmd
from concourse.masks import make_identity
from concourse.tile import TileContext

F32 = mybir.dt.float32
F16 = mybir.dt.float16

NCORES = 8
B, N, C = 64, 320, 768
H, DH = 12, 64
KS = C // 128  # 6 contraction subtiles
B_CORE = B // NCORES  # 8 batches per core
PAIR_TOK = 2 * N  # 640
NPAIR = B_CORE // 2  # 4
TOK_CORE = B_CORE * N  # 2560
SLOT = 85  # psum column stride per head in PV output (6 heads/bank)

# key chunks of one batch's 320 tokens (partition base 0 each)
KT_CHUNKS = [(0, 128), (128, 128), (256, 64)]
# query chunks: template | search 0:128 | search 128:256
Q_CHUNKS = [(0, 64), (64, 128), (192, 128)]
# token chunks used for the projection / output rows
P_CHUNKS = [(0, 128), (128, 128), (256, 64)]


def _emit_pair(nc, tc, pools, consts, x_ap, out_ap, p):
    """Emit instructions for one pair of batches (640 tokens)."""
    psum = pools["psum"]
    psum_b = pools["psum_b"]
    wqkv16, wproj16, bias_bc, ident32, ident16 = consts

    # ---- load x pair and build x^T (fp16, C on partitions) ----
    x_nat = pools["x_nat"].tile([128, 5, C], F32, tag="x_nat")
    nc.sync.dma_start(
        x_nat[:],
        x_ap[p * PAIR_TOK : (p + 1) * PAIR_TOK, :].rearrange(
            "(t p) f -> p t f", p=128
        ),
    )
    xT = pools["xT"].tile([128, KS, PAIR_TOK], F16, tag="xT")
    for fc in range(KS):
        ps_a = psum.tile([128, 512], F32, tag="ps")
        ps_b = psum.tile([128, 512], F32, tag="ps")
        for t in range(5):
            dst = ps_a[:, t * 128 : (t + 1) * 128] if t < 4 else ps_b[:, 0:128]
            nc.tensor.transpose(
                dst, x_nat[:, t, fc * 128 : (fc + 1) * 128], ident32
            )
        nc.vector.tensor_copy(xT[:, fc, 0:512], ps_a[:, 0:512])
        nc.vector.tensor_copy(xT[:, fc, 512:640], ps_b[:, 0:128])

    # ---- q^T / k^T : out^T = W_qk.T @ x^T, features on partitions ----
    # feature chunk fc covers q (fc 0..5) then k (fc 6..11); head pair per chunk
    qkT = pools["qkT"].tile([128, 2 * KS, PAIR_TOK], F16, tag="qkT")
    qkTo = pools["qkTo"].tile([64, 2 * KS, PAIR_TOK], F16, tag="qkTo")
    for fc in range(2 * KS):
        ps1 = psum.tile([128, 512], F32, tag="ps")
        ps2 = psum.tile([128, 512], F32, tag="ps")
        for ks in range(KS):
            lhsT = wqkv16[:, ks, fc * 128 : (fc + 1) * 128]
            nc.tensor.matmul(
                ps1[:, 0:512],
                lhsT=lhsT,
                rhs=xT[:, ks, 0:512],
                start=(ks == 0),
                stop=(ks == KS - 1),
            )
            nc.tensor.matmul(
                ps2[:, 0:128],
                lhsT=lhsT,
                rhs=xT[:, ks, 512:640],
                start=(ks == 0),
                stop=(ks == KS - 1),
            )
        nc.scalar.copy(qkT[:, fc, 0:512], ps1[:, 0:512])
        nc.scalar.copy(qkT[:, fc, 512:640], ps2[:, 0:128])
        # odd head of this chunk lives at partitions 64..128; shift it
        # to partition base 0 (matmul operands must not sit at base 64)
        nc.sync.dma_start(qkTo[0:64, fc, :], qkT[64:128, fc, :])

    # ---- per batch within the pair ----
    for b2 in range(2):
        bb = p * 2 + b2  # batch index within this core
        btok = b2 * N

        # ---- v natural [tok, h, 0:65] with ones column ----
        va = pools["v"].tile([128, 3, H, 66], F16, tag="v")
        for ci, (off, sz) in enumerate(KT_CHUNKS):
            for half, (n0, nw) in enumerate([(0, 512), (512, 256)]):
                ps = psum.tile([128, 512], F32, tag="ps")
                for ks in range(KS):
                    nc.tensor.matmul(
                        ps[:sz, 0:nw],
                        lhsT=xT[:, ks, btok + off : btok + off + sz],
                        rhs=wqkv16[:, ks, 2 * C + n0 : 2 * C + n0 + nw],
                        start=(ks == 0),
                        stop=(ks == KS - 1),
                    )
                h0, nh = (0, 8) if half == 0 else (8, 4)
                nc.vector.tensor_copy(
                    va[:sz, ci, h0 : h0 + nh, 0:64],
                    ps[:sz, 0:nw].rearrange("p (h d) -> p h d", d=64),
                )
        nc.vector.memset(va[:, :, :, 64], 1.0)

        def kT(h):
            src = qkT if h % 2 == 0 else qkTo
            return src[0:64, KS + h // 2, :]

        def qT(h):
            src = qkT if h % 2 == 0 else qkTo
            return src[0:64, h // 2, :]

        # ---- search scores S^T [kt, q] + exp (4 heads per 2-bank psum) ----
        es_ci = []
        for ci, (off, sz) in enumerate(KT_CHUNKS):
            es = pools["expS"].tile([128, H, 256], F16, tag="expS")
            es_ci.append(es)
            for hg in range(3):
                ps = psum_b.tile([128, 1024], F32, tag="psb")
                for j in range(4):
                    h = 4 * hg + j
                    nc.tensor.matmul(
                        ps[:sz, j * 256 : (j + 1) * 256],
                        lhsT=kT(h)[:, btok + off : btok + off + sz],
                        rhs=qT(h)[:, btok + 64 : btok + 320],
                        start=True,
                        stop=True,
                    )
                nc.scalar.activation(
                    es[:sz, 4 * hg : 4 * hg + 4, :],
                    ps[:sz, 0:1024].rearrange("p (h q) -> p h q", q=256),
                    mybir.ActivationFunctionType.Exp,
                    scale=0.125,
                )

        # ---- template scores (template attends only to template keys) ----
        esm = pools["expSmt"].tile([64, H, 64], F16, tag="expSmt")
        for hg in range(2):
            ps = psum.tile([128, 512], F32, tag="ps")
            for j in range(6):
                h = 6 * hg + j
                nc.tensor.matmul(
                    ps[0:64, j * 64 : (j + 1) * 64],
                    lhsT=kT(h)[:, btok : btok + 64],
                    rhs=qT(h)[:, btok : btok + 64],
                    start=True,
                    stop=True,
                )
            nc.scalar.activation(
                esm[:, 6 * hg : 6 * hg + 6, :],
                ps[0:64, 0:384].rearrange("p (h q) -> p h q", q=64),
                mybir.ActivationFunctionType.Exp,
                scale=0.125,
            )

        # ---- PV + softmax normalization -> attn rows (fp16) ----
        # attn chunk qg holds rows [Q_CHUNKS[qg]] of the batch at base 0
        attn = pools["attn"].tile([128, 3, C], F16, tag="attn")
        nc.vector.memset(attn[64:128, 0, :], 0.0)  # pad rows read by DMA-T

        def normalize(po, qsz, qg, half):
            po_v = po[:qsz, 0:510].rearrange("p (h s) -> p h s", s=SLOT)
            rcp = pools["rcp"].tile([128, 8], F32, tag="rcp")
            nc.vector.reciprocal(rcp[:qsz, 0:6], po_v[:, :, 64])
            nc.vector.tensor_tensor(
                attn[:qsz, qg, half * 384 : (half + 1) * 384].rearrange(
                    "p (h d) -> p h d", d=64
                ),
                po_v[:, :, 0:64],
                rcp[:qsz, 0:6, None].to_broadcast([qsz, 6, 64]),
                mybir.AluOpType.mult,
            )

        for half in range(2):
            # template rows (batch rows 0..64)
            po = psum.tile([128, 512], F32, tag="ps")
            for j in range(6):
                h = 6 * half + j
                nc.tensor.matmul(
                    po[0:64, j * SLOT : j * SLOT + 65],
                    lhsT=esm[:, h, 0:64],
                    rhs=va[0:64, 0, h, 0:65],
                    start=True,
                    stop=True,
                )
            normalize(po, 64, 0, half)
            # search rows: q chunks of 128
            for qg in (1, 2):
                qlo = Q_CHUNKS[qg][0] - 64
                po = psum.tile([128, 512], F32, tag="ps")
                for j in range(6):
                    h = 6 * half + j
                    for ci, (koff, ksz) in enumerate(KT_CHUNKS):
                        nc.tensor.matmul(
                            po[0:128, j * SLOT : j * SLOT + 65],
                            lhsT=es_ci[ci][:ksz, h, qlo : qlo + 128],
                            rhs=va[:ksz, ci, h, 0:65],
                            start=(ci == 0),
                            stop=(ci == 2),
                        )
                normalize(po, 128, qg, half)

        # ---- attn^T via PE transposes (fp16) ----
        attnT = pools["attnT"].tile([128, KS, N], F16, tag="attnT")
        for fc in range(KS):
            pt = pools["psum_h"].tile([128, 512], F16, tag="psh")
            for qg, dst0 in ((0, 0), (1, 64), (2, 192)):
                qsz = Q_CHUNKS[qg][1]
                nc.tensor.transpose(
                    pt[:, dst0 : dst0 + qsz],
                    attn[0:qsz, qg, fc * 128 : (fc + 1) * 128],
                    ident16[:qsz, :qsz],
                )
            nc.vector.tensor_copy(attnT[:, fc, 0:N], pt[:, 0:N])

        # ---- output projection + bias ----
        out_sb = pools["out"].tile([128, 3, C], F32, tag="out")
        for qc, (qoff, qsz) in enumerate(P_CHUNKS):
            for half, (n0, nw) in enumerate([(0, 512), (512, 256)]):
                pp = psum.tile([128, 512], F32, tag="ps")
                for ks in range(KS):
                    nc.tensor.matmul(
                        pp[:qsz, 0:nw],
                        lhsT=attnT[:, ks, qoff : qoff + qsz],
                        rhs=wproj16[:, ks, n0 : n0 + nw],
                        start=(ks == 0),
                        stop=(ks == KS - 1),
                    )
                nc.vector.tensor_tensor(
                    out_sb[:qsz, qc, n0 : n0 + nw],
                    pp[:qsz, 0:nw],
                    bias_bc[:qsz, n0 : n0 + nw],
                    mybir.AluOpType.add,
                )
        row0 = bb * N
        nc.sync.dma_start(
            out_ap[row0 : row0 + 256, :].rearrange("(t p) f -> p t f", p=128),
            out_sb[:, 0:2, :],
        )
        nc.sync.dma_start(out_ap[row0 + 256 : row0 + N, :], out_sb[0:64, 2, :])


def build_kernel():
    nc = bacc.Bacc("TRN2", target_bir_lowering=False)
    x_t = nc.dram_tensor("x", [TOK_CORE, C], F32, kind="ExternalInput")
    wqkv_t = nc.dram_tensor("W_qkv", [C, 3 * C], F32, kind="ExternalInput")
    wproj_t = nc.dram_tensor("W_proj", [C, C], F32, kind="ExternalInput")
    bias_t = nc.dram_tensor("b_proj", [C], F32, kind="ExternalInput")
    out_t = nc.dram_tensor("out", [TOK_CORE, C], F32, kind="ExternalOutput")

    with TileContext(nc) as tc:
        import contextlib

        with contextlib.ExitStack() as ctx:
            pools = {
                "const": ctx.enter_context(tc.tile_pool(name="const", bufs=1)),
                "stage": ctx.enter_context(tc.tile_pool(name="stage", bufs=2)),
                "x_nat": ctx.enter_context(tc.tile_pool(name="x_nat", bufs=1)),
                "xT": ctx.enter_context(tc.tile_pool(name="xT", bufs=2)),
                "qkT": ctx.enter_context(tc.tile_pool(name="qkT", bufs=2)),
                "qkTo": ctx.enter_context(tc.tile_pool(name="qkTo", bufs=2)),
                "v": ctx.enter_context(tc.tile_pool(name="v", bufs=2)),
                "expS": ctx.enter_context(tc.tile_pool(name="expS", bufs=3)),
                "expSmt": ctx.enter_context(tc.tile_pool(name="expSmt", bufs=2)),
                "attn": ctx.enter_context(tc.tile_pool(name="attn", bufs=2)),
                "attnT": ctx.enter_context(tc.tile_pool(name="attnT", bufs=2)),
                "out": ctx.enter_context(tc.tile_pool(name="out", bufs=1)),
                "rcp": ctx.enter_context(tc.tile_pool(name="rcp", bufs=4)),
                "psum": ctx.enter_context(
                    tc.tile_pool(name="psum", bufs=3, space="PSUM")
                ),
                "psum_h": ctx.enter_context(
                    tc.tile_pool(name="psum_h", bufs=1, space="PSUM")
                ),
                "psum_b": ctx.enter_context(
                    tc.tile_pool(name="psum_b", bufs=2, space="PSUM")
                ),
            }
            const = pools["const"]

            # constants: fp16 weights, broadcast bias, identity
            wqkv16 = const.tile([128, KS, 3 * C], F16, tag="wqkv16")
            wproj16 = const.tile([128, KS, C], F16, tag="wproj16")
            bias_bc = const.tile([128, C], F32, tag="bias_bc")
            ident32 = const.tile([128, 128], F32, tag="ident32")
            ident16 = const.tile([128, 128], F16, tag="ident16")
            make_identity(nc, ident32)
            make_identity(nc, ident16)

            for ks in range(KS):
                st = pools["stage"].tile([128, 3 * C], F32, tag="stage")
                nc.sync.dma_start(st[:], wqkv_t.ap()[ks * 128 : (ks + 1) * 128, :])
                nc.vector.tensor_copy(wqkv16[:, ks, :], st[:])
            for ks in range(KS):
                st = pools["stage"].tile([128, 3 * C], F32, tag="stage")
                nc.sync.dma_start(
                    st[:, 0:C], wproj_t.ap()[ks * 128 : (ks + 1) * 128, :]
                )
                nc.vector.tensor_copy(wproj16[:, ks, :], st[:, 0:C])
            brow = pools["stage"].tile([128, 3 * C], F32, tag="stage")
            nc.sync.dma_start(brow[0:1, 0:C], bias_t.ap().unsqueeze(0))
            nc.gpsimd.partition_broadcast(bias_bc[:, :], brow[0:1, 0:C])

            consts = (wqkv16, wproj16, bias_bc, ident32, ident16)
            for p in range(NPAIR):
                _emit_pair(nc, tc, pools, consts, x_t.ap(), out_t.ap(), p)

    nc.compile()
    return nc


@functools.cache
def _get_nc():
    return build_kernel()


def kernel(**inputs):
    x = np.ascontiguousarray(np.asarray(inputs["x"], dtype=np.float32))
    wqkv = np.ascontiguousarray(np.asarray(inputs["W_qkv"], dtype=np.float32))
    wproj = np.ascontiguousarray(np.asarray(inputs["W_proj"], dtype=np.float32))
    bias = np.ascontiguousarray(np.asarray(inputs["b_proj"], dtype=np.float32))
    t_h = int(inputs.get("t_h", 8))
    t_w = int(inputs.get("t_w", 8))
    assert t_h * t_w == 64, "kernel built for template length 64"
    assert x.shape == (B, N, C)

    nc = _get_nc()
    in_maps = [
        {
            "x": x[c * B_CORE : (c + 1) * B_CORE].reshape(TOK_CORE, C),
            "W_qkv": wqkv,
            "W_proj": wproj,
            "b_proj": bias,
        }
        for c in range(NCORES)
    ]
    res = run_bass_kernel_spmd(nc, in_maps, core_ids=list(range(NCORES)))
    out = np.concatenate(
        [r["out"].reshape(B_CORE, N, C) for r in res.results], axis=0
    )
    return out.astype(np.float32)


if __name__ == "__main__":
    _get_nc()
    print("kernel built OK")

